# revision 1
# baseline (speedup 1.0000x reference)
"""Trainium2 Bass kernel for nn_AbsDiagNet: out = scan(|p_t + c*h|) @ W_ho.T + b_ho.

Algorithm
---------
reference:  pre = einsum('sbi,hi->sbh', X, W_ih)          # big GEMM
            h_{t+1} = |pre[t] + c * h_t|, h_0 = 0         # serial scan, c=hh[0]
            out = h_S @ W_ho.T + b_ho

Key transform: abs is positively homogeneous, so with g_t := h_t * c^{-t}:
            g_{t+1} = | g_t + pre[t] * c^{-(t+1)} |
The per-step multiply disappears. We fold -c^{-(t+1)} into X on the host, so
the device GEMM directly produces  qn[t] = -pre[t]*c^{-(t+1)},  and the scan is
            g_{t+1} = | g_t - qn[t] |   (ABSOLUTE_DIFF)
which is a single-ALU-op recurrence that a custom DVE op runs at
1 element/lane/cycle along the free axis. h_S = c^S * g_S is folded into W_ho.

Sharding: data-parallel over batch B=128 -> 16 per core x 8 cores. X is
host-transposed per core to [I, Bc, S] so the GEMM needs no on-device
transposes (contraction dim I lands on partitions). GEMM runs in float32r
(full-rate fp32, ~12 mantissa bits). Scan state/output in fp32.
"""
import os
import numpy as np
from contextlib import ExitStack

import concourse.bass as bass
import concourse.tile as tile
from concourse import bacc, mybir
import concourse.bass_utils as bass_utils
import concourse.dve_ops as dve_ops_mod
from concourse.dve_ops import DveOp
from concourse.dve_spec import (
    Spec, Scan as SpecScan, AluOp as DAlu, Src0, Zero, lower as dve_lower,
    _has_src1,
)
from concourse.dve_uop import DveOpSpec

F32 = mybir.dt.float32
F32R = mybir.dt.float32r
Act = mybir.ActivationFunctionType

S, B, I, H, O = 2048, 128, 256, 1024, 256
NCORES = 8
BC = B // NCORES          # 16 batches per core
HB = H // 128             # 8 h-blocks
NTC = S // 512            # 4 matmul t-chunks of 512

# --- disable walrus birsim (verification-only; big compile-time cost) -------
_orig_run_command = bass_utils.run_command


def _run_command_nobirsim(argv, **kw):
    argv = ["--enable-birsim=false" if a == "--enable-birsim=true" else a
            for a in argv]
    return _orig_run_command(argv, **kw)


bass_utils.run_command = _run_command_nobirsim


# --- custom DVE op: inclusive scan with state = |state - x| ------------------
def _register_scan_op() -> DveOp:
    name = "ABS_DIFF_SCAN_ANT"
    if name in dve_ops_mod._SUB_OPCODE_FOR_NAME:
        return next(o for o in dve_ops_mod.OPS if o.name == name)
    spec = Spec(body=SpecScan(DAlu.ABSOLUTE_DIFF, Src0, init=Zero))
    row = max(dve_ops_mod._SUB_OPCODE_FOR_NAME.values()) + 1
    assert row < 0x20
    dve_ops_mod._SUB_OPCODE_FOR_NAME[name] = row
    sha = DveOpSpec(name=name, opcode=row, uops=dve_lower(spec, ver="v3"),
                    rd1_en=_has_src1(spec)).sha("v3")
    op = DveOp(name, spec, subdim=False, uops_sha={"v3": sha})
    dve_ops_mod.OPS.append(op)
    dve_ops_mod.CUSTOM_DVE_SPECS[name] = spec
    return op


_SCAN_OP = _register_scan_op()
_BUILD_CACHE: dict = {}


def _build(repeat: int = 1):
    """Build + compile the per-core Bass module (same NEFF on all 8 cores)."""
    if repeat in _BUILD_CACHE:
        return _BUILD_CACHE[repeat]
    nc = bacc.Bacc("TRN2", target_bir_lowering=False, debug=False)
    X = nc.dram_tensor("X", [I, BC, S], F32R, kind="ExternalInput").ap()
    WT = nc.dram_tensor("WT", [I, H], F32R, kind="ExternalInput").ap()
    WHO = nc.dram_tensor("WHO", [H, O], F32, kind="ExternalInput").ap()
    BIA = nc.dram_tensor("BIA", [128, O // 128], F32, kind="ExternalInput").ap()
    OUT = nc.dram_tensor("out", [O, BC], F32, kind="ExternalOutput").ap()

    with tile.TileContext(nc) as tc, ExitStack() as ctx:
        const = ctx.enter_context(tc.tile_pool(name="const", bufs=1))
        xp = ctx.enter_context(tc.tile_pool(name="xp", bufs=2))
        qp = ctx.enter_context(tc.tile_pool(name="qp", bufs=2))
        pp = ctx.enter_context(tc.tile_pool(name="pp", bufs=3, space="PSUM"))
        pf = ctx.enter_context(tc.tile_pool(name="pf", bufs=1, space="PSUM"))
        outp = ctx.enter_context(tc.tile_pool(name="outp", bufs=1))

        wt0 = const.tile([128, H], F32R, tag="wt0")
        nc.sync.dma_start(wt0[:], WT[0:128, :])
        wt1 = const.tile([128, H], F32R, tag="wt1")
        nc.sync.dma_start(wt1[:], WT[128:256, :])
        who = const.tile([128, HB * O], F32, tag="who")
        nc.sync.dma_start(
            who[:].rearrange("p (g o) -> p g o", g=HB, o=O),
            WHO.rearrange("(g p) o -> p g o", p=128),
        )
        bia = const.tile([128, O // 128], F32, tag="bia")
        nc.sync.dma_start(bia[:], BIA[:])

        for _rep in range(repeat):
            h_all = outp.tile([128, HB * BC], F32, tag="h_all")
            for b in range(BC):
                x0 = xp.tile([128, S], F32R, tag="x0")
                nc.sync.dma_start(x0[:], X[0:128, b, :])
                x1 = xp.tile([128, S], F32R, tag="x1")
                nc.sync.dma_start(x1[:], X[128:256, b, :])

                q = qp.tile([128, HB * S], F32, tag="q")
                for hb in range(HB):
                    for half in range(2):          # two [128,1024] psum tiles per hb
                        pt = pp.tile([128, 1024], F32, tag="ps")
                        for tci in range(2):       # two 512-col matmul groups
                            sl_p = slice(tci * 512, tci * 512 + 512)
                            tc_g = half * 2 + tci
                            sl_x = slice(tc_g * 512, tc_g * 512 + 512)
                            nc.tensor.matmul(pt[:, sl_p],
                                             wt0[:, hb * 128:(hb + 1) * 128],
                                             x0[:, sl_x], start=True, stop=False)
                            nc.tensor.matmul(pt[:, sl_p],
                                             wt1[:, hb * 128:(hb + 1) * 128],
                                             x1[:, sl_x], start=False, stop=True)
                        nc.scalar.copy(
                            q[:, hb * S + half * 1024: hb * S + half * 1024 + 1024],
                            pt[:])
                # 8 scans, one per h-block, in place: g = |g_prev - qn_t| along t
                for hb in range(HB):
                    sl = slice(hb * S, (hb + 1) * S)
                    nc.vector._custom_dve(_SCAN_OP, out=q[:, sl], in0=q[:, sl])
                # gather final g_S of each h-block into h_all[:, (hb, b)]
                qv = q[:].rearrange("p (g s) -> p g s", g=HB, s=S)
                hv = h_all[:].rearrange("p (g b) -> p g b", g=HB, b=BC)
                nc.scalar.copy(hv[:, :, b], qv[:, :, S - 1])

            # output projection: out[o, b] = sum_h WHO[h, o] * h_all[h, b] + bias
            for ob in range(O // 128):
                po = pf.tile([128, BC], F32, tag="po")
                for hb in range(HB):
                    lhs = who[:, hb * O + ob * 128: hb * O + ob * 128 + 128]
                    rhs = h_all[:, hb * BC:(hb + 1) * BC]
                    nc.tensor.matmul(po[:], lhs, rhs,
                                     start=(hb == 0), stop=(hb == HB - 1))
                ot = outp.tile([128, BC], F32, tag=f"ot{ob}")
                nc.scalar.activation(ot[:], po[:], Act.Identity,
                                     bias=bia[:, ob:ob + 1], scale=1.0)
                nc.sync.dma_start(OUT[ob * 128:(ob + 1) * 128, :], ot[:])

    nc.compile()
    _BUILD_CACHE[repeat] = nc
    return nc


def _prep_inputs(X, W_ih, hh, W_ho, b_ho):
    X = np.asarray(X, dtype=np.float32)
    W_ih = np.asarray(W_ih, dtype=np.float32)
    hh = np.asarray(hh, dtype=np.float32).reshape(-1)
    W_ho = np.asarray(W_ho, dtype=np.float32)
    b_ho = np.asarray(b_ho, dtype=np.float32).reshape(-1)
    c = float(hh[0])
    assert np.allclose(hh, c), "kernel assumes uniform hh (setup_inputs gives 0.99)"
    assert 0.0 < c, "scan rescaling requires positive hh"

    # fold -c^{-(t+1)} into X; fold c^S into W_ho
    tscale = (-np.power(np.float64(c), -(np.arange(S, dtype=np.float64) + 1.0))
              ).astype(np.float32)
    Xs = X * tscale[:, None, None]

    WT_h = np.ascontiguousarray(W_ih.T)                                   # [I, H]
    WHO_h = np.ascontiguousarray((W_ho * np.float32(c ** S)).T)           # [H, O]
    BIA_h = np.ascontiguousarray(b_ho.reshape(O // 128, 128).T)           # [128, 2]

    in_maps = []
    for k in range(NCORES):
        xc = np.ascontiguousarray(
            Xs[:, k * BC:(k + 1) * BC, :].transpose(2, 1, 0))             # [I, BC, S]
        in_maps.append(dict(X=xc, WT=WT_h, WHO=WHO_h, BIA=BIA_h))
    return in_maps


def _run(nc, in_maps):
    res = bass_utils.run_bass_kernel_spmd(nc, in_maps, core_ids=list(range(NCORES)))
    return np.concatenate(
        [res.results[k]["out"].T for k in range(NCORES)], axis=0)        # [B, O]


def kernel(X, W_ih, hh, W_ho, b_ho):
    in_maps = _prep_inputs(X, W_ih, hh, W_ho, b_ho)
    nc = _build(repeat=1)
    return _run(nc, in_maps).astype(np.float32)


# revision 6
# speedup vs baseline: 343.0959x; 343.0959x over previous
"""Trainium2 Bass kernel for nn_AbsDiagNet: out = scan(|p_t + c*h|) @ W_ho.T + b_ho.

Algorithm
---------
reference:  pre = einsum('sbi,hi->sbh', X, W_ih)          # big GEMM
            h_{t+1} = |pre[t] + c * h_t|, h_0 = 0         # serial scan, c=hh[0]
            out = h_S @ W_ho.T + b_ho

Key transform: abs is positively homogeneous, so with g_t := h_t * c^{-t}:
            g_{t+1} = | g_t + pre[t] * c^{-(t+1)} |
The per-step multiply disappears. We fold -c^{-(t+1)} into X on the host, so
the device GEMM directly produces  qn[t] = -pre[t]*c^{-(t+1)},  and the scan is
            g_{t+1} = | g_t - qn[t] |   (ABSOLUTE_DIFF)
which is a single-ALU-op recurrence that a custom DVE op runs at
1 element/lane/cycle along the free axis. h_S = c^S * g_S is folded into W_ho.

Sharding: data-parallel over batch B=128 -> 16 per core x 8 cores. X is
host-transposed per core to [I, Bc, S] so the GEMM needs no on-device
transposes (contraction dim I lands on partitions). GEMM runs in float32r
(full-rate fp32, ~12 mantissa bits). Scan state/output in fp32.
"""
import os
import numpy as np
from contextlib import ExitStack

import concourse.bass as bass
import concourse.tile as tile
from concourse import bacc, mybir
import concourse.bass_utils as bass_utils
import concourse.dve_ops as dve_ops_mod
from concourse.dve_ops import DveOp
from concourse.dve_spec import (
    Spec, Scan as SpecScan, AluOp as DAlu, Src0, Zero, lower as dve_lower,
    _has_src1,
)
from concourse.dve_uop import DveOpSpec

F32 = mybir.dt.float32
F32R = mybir.dt.float32r
BF16 = mybir.dt.bfloat16
Act = mybir.ActivationFunctionType

# GEMM input dtype: "f32r" (~12 mantissa bits, end-to-end relerr ~1.4e-4) or
# "bf16" (8 bits, relerr ~2.3e-3, slightly faster PE + half the X DMA traffic).
# Both run at ~300-360us/core (DVE-scan-bound); f32r kept for accuracy margin.
GEMM_DTYPE = "f32r"

S, B, I, H, O = 2048, 128, 256, 1024, 256
NCORES = 8
BC = B // NCORES          # 16 batches per core
HB = H // 128             # 8 h-blocks
NTC = S // 512            # 4 matmul t-chunks of 512

# --- disable walrus birsim (verification-only; big compile-time cost) -------
_orig_run_command = bass_utils.run_command


def _run_command_nobirsim(argv, **kw):
    argv = ["--enable-birsim=false" if a == "--enable-birsim=true" else a
            for a in argv]
    return _orig_run_command(argv, **kw)


bass_utils.run_command = _run_command_nobirsim


# --- custom DVE op: inclusive scan with state = |state - x| ------------------
def _register_scan_op() -> DveOp:
    name = "ABS_DIFF_SCAN_ANT"
    if name in dve_ops_mod._SUB_OPCODE_FOR_NAME:
        return next(o for o in dve_ops_mod.OPS if o.name == name)
    spec = Spec(body=SpecScan(DAlu.ABSOLUTE_DIFF, Src0, init=Zero))
    row = max(dve_ops_mod._SUB_OPCODE_FOR_NAME.values()) + 1
    assert row < 0x20
    dve_ops_mod._SUB_OPCODE_FOR_NAME[name] = row
    sha = DveOpSpec(name=name, opcode=row, uops=dve_lower(spec, ver="v3"),
                    rd1_en=_has_src1(spec)).sha("v3")
    op = DveOp(name, spec, subdim=False, uops_sha={"v3": sha})
    dve_ops_mod.OPS.append(op)
    dve_ops_mod.CUSTOM_DVE_SPECS[name] = spec
    return op


_SCAN_OP = _register_scan_op()
_BUILD_CACHE: dict = {}


def _build(repeat: int = 1):
    """Build + compile the per-core Bass module (same NEFF on all 8 cores)."""
    if repeat in _BUILD_CACHE:
        return _BUILD_CACHE[repeat]
    GD = BF16 if GEMM_DTYPE == "bf16" else F32R
    nc = bacc.Bacc("TRN2", target_bir_lowering=False, debug=False)
    X = nc.dram_tensor("X", [I, BC, S], GD, kind="ExternalInput").ap()
    WT = nc.dram_tensor("WT", [I, H], GD, kind="ExternalInput").ap()
    WHO = nc.dram_tensor("WHO", [H, O], F32, kind="ExternalInput").ap()
    BIA = nc.dram_tensor("BIA", [128, O // 128], F32, kind="ExternalInput").ap()
    OUT = nc.dram_tensor("out", [O, BC], F32, kind="ExternalOutput").ap()

    with tile.TileContext(nc) as tc, ExitStack() as ctx:
        const = ctx.enter_context(tc.tile_pool(name="const", bufs=1))
        xp = ctx.enter_context(tc.tile_pool(name="xp", bufs=3))
        gp = ctx.enter_context(tc.tile_pool(name="gp", bufs=2))
        pp = ctx.enter_context(tc.tile_pool(name="pp", bufs=2, space="PSUM"))
        outp = ctx.enter_context(tc.tile_pool(name="outp", bufs=1))

        wt0 = const.tile([128, H], GD, tag="wt0")
        nc.sync.dma_start(wt0[:], WT[0:128, :])
        wt1 = const.tile([128, H], GD, tag="wt1")
        nc.sync.dma_start(wt1[:], WT[128:256, :])
        who = const.tile([128, HB * O], F32, tag="who")
        nc.sync.dma_start(
            who[:].rearrange("p (g o) -> p g o", g=HB, o=O),
            WHO.rearrange("(g p) o -> p g o", p=128),
        )
        bia = const.tile([128, O // 128], F32, tag="bia")
        nc.sync.dma_start(bia[:], BIA[:])

        for _rep in range(repeat):
            h_all = outp.tile([128, HB * BC], F32, tag="h_all")
            for b in range(BC):
                x0 = xp.tile([128, S], GD, tag="x0")
                nc.sync.dma_start(x0[:], X[0:128, b, :])
                x1 = xp.tile([128, S], GD, tag="x1")
                nc.sync.dma_start(x1[:], X[128:256, b, :])

                for hb in range(HB):
                    # qn for (b, hb): [128 h_sub, 2048 t] accumulated in PSUM
                    pt = pp.tile([128, S], F32, tag="ps")
                    for tci in range(NTC):
                        sl = slice(tci * 512, tci * 512 + 512)
                        nc.tensor.matmul(pt[:, sl],
                                         wt0[:, hb * 128:(hb + 1) * 128],
                                         x0[:, sl], start=True, stop=False)
                        nc.tensor.matmul(pt[:, sl],
                                         wt1[:, hb * 128:(hb + 1) * 128],
                                         x1[:, sl], start=False, stop=True)
                    # scan straight out of PSUM: g_t = |g_{t-1} - qn_t| along t
                    g = gp.tile([128, S], F32, tag="g")
                    nc.vector._custom_dve(_SCAN_OP, out=g[:], in0=pt[:])
                    # keep only g_S
                    nc.scalar.copy(h_all[:, hb * BC + b: hb * BC + b + 1],
                                   g[:, S - 1:S])

            # output projection: out[o, b] = sum_h WHO[h, o] * h_all[h, b] + bias
            for ob in range(O // 128):
                po = pp.tile([128, BC], F32, tag="ps")
                for hb in range(HB):
                    lhs = who[:, hb * O + ob * 128: hb * O + ob * 128 + 128]
                    rhs = h_all[:, hb * BC:(hb + 1) * BC]
                    nc.tensor.matmul(po[:], lhs, rhs,
                                     start=(hb == 0), stop=(hb == HB - 1))
                ot = outp.tile([128, BC], F32, tag=f"ot{ob}")
                nc.scalar.activation(ot[:], po[:], Act.Identity,
                                     bias=bia[:, ob:ob + 1], scale=1.0)
                nc.sync.dma_start(OUT[ob * 128:(ob + 1) * 128, :], ot[:])

    nc.compile()
    _BUILD_CACHE[repeat] = nc
    return nc


def _prep_inputs(X, W_ih, hh, W_ho, b_ho):
    X = np.asarray(X, dtype=np.float32)
    W_ih = np.asarray(W_ih, dtype=np.float32)
    hh = np.asarray(hh, dtype=np.float32).reshape(-1)
    W_ho = np.asarray(W_ho, dtype=np.float32)
    b_ho = np.asarray(b_ho, dtype=np.float32).reshape(-1)
    c = float(hh[0])
    assert np.allclose(hh, c), "kernel assumes uniform hh (setup_inputs gives 0.99)"
    assert 0.0 < c, "scan rescaling requires positive hh"

    # fold -c^{-(t+1)} into X; fold c^S into W_ho
    tscale = (-np.power(np.float64(c), -(np.arange(S, dtype=np.float64) + 1.0))
              ).astype(np.float32)
    Xs = X * tscale[:, None, None]

    if GEMM_DTYPE == "bf16":
        import ml_dtypes
        gnp = ml_dtypes.bfloat16
        Xs = Xs.astype(gnp)
        WT_h = np.ascontiguousarray(W_ih.T.astype(gnp))                   # [I, H]
    else:
        WT_h = np.ascontiguousarray(W_ih.T)                               # [I, H]
    WHO_h = np.ascontiguousarray((W_ho * np.float32(c ** S)).T)           # [H, O]
    BIA_h = np.ascontiguousarray(b_ho.reshape(O // 128, 128).T)           # [128, 2]

    in_maps = []
    for k in range(NCORES):
        xc = np.ascontiguousarray(
            Xs[:, k * BC:(k + 1) * BC, :].transpose(2, 1, 0))             # [I, BC, S]
        in_maps.append(dict(X=xc, WT=WT_h, WHO=WHO_h, BIA=BIA_h))
    return in_maps


def _run(nc, in_maps):
    res = bass_utils.run_bass_kernel_spmd(nc, in_maps, core_ids=list(range(NCORES)))
    return np.concatenate(
        [res.results[k]["out"].T for k in range(NCORES)], axis=0)        # [B, O]


def kernel(X, W_ih, hh, W_ho, b_ho):
    in_maps = _prep_inputs(X, W_ih, hh, W_ho, b_ho)
    nc = _build(repeat=1)
    return _run(nc, in_maps).astype(np.float32)


# revision 9
# speedup vs baseline: 415.4790x; 1.2110x over previous
"""Trainium2 Bass kernel for nn_AbsDiagNet: out = scan(|p_t + c*h|) @ W_ho.T + b_ho.

Algorithm
---------
reference:  pre = einsum('sbi,hi->sbh', X, W_ih)          # big GEMM
            h_{t+1} = |pre[t] + c * h_t|, h_0 = 0         # serial scan, c=hh[0]
            out = h_S @ W_ho.T + b_ho

Key transform: abs is positively homogeneous, so with g_t := h_t * c^{-t}:
            g_{t+1} = | g_t + pre[t] * c^{-(t+1)} |
The per-step multiply disappears. We fold -c^{-(t+1)} into X on the host, so
the device GEMM directly produces  qn[t] = -pre[t]*c^{-(t+1)},  and the scan is
            g_{t+1} = | g_t - qn[t] |   (ABSOLUTE_DIFF)
which is a single-ALU-op recurrence that a custom DVE op runs at
1 element/lane/cycle along the free axis. h_S = c^S * g_S is folded into W_ho.

Sharding: data-parallel over batch B=128 -> 16 per core x 8 cores. X is
host-transposed per core to [I, Bc, S] so the GEMM needs no on-device
transposes (contraction dim I lands on partitions). GEMM inputs are bf16
(see GEMM_DTYPE); PSUM accumulation and the scan are fp32.
"""
import os
import numpy as np
from contextlib import ExitStack

import concourse.bass as bass
import concourse.tile as tile
from concourse import bacc, mybir
import concourse.bass_utils as bass_utils
import concourse.dve_ops as dve_ops_mod
from concourse.dve_ops import DveOp
from concourse.dve_spec import (
    Spec, Scan as SpecScan, AluOp as DAlu, Src0, Zero, lower as dve_lower,
    _has_src1,
)
from concourse.dve_uop import DveOpSpec

F32 = mybir.dt.float32
F32R = mybir.dt.float32r
BF16 = mybir.dt.bfloat16
Act = mybir.ActivationFunctionType

# GEMM input dtype: "bf16" (end-to-end relerr ~2.3e-3, ~296us/core) or
# "f32r" (~12 mantissa bits, relerr ~1.4e-4, ~330us/core). Both are far inside
# the 2e-2 gate; the wall is the DVE scan floor (~292us cost-model) either
# way, but bf16's fast FWL weight loads keep the 1024 matmuls fully hidden.
GEMM_DTYPE = "bf16"

S, B, I, H, O = 2048, 128, 256, 1024, 256
NCORES = 8
BC = B // NCORES          # 16 batches per core
HB = H // 128             # 8 h-blocks
NTC = S // 512            # 4 matmul t-chunks of 512

# --- disable walrus birsim (verification-only; big compile-time cost) -------
_orig_run_command = bass_utils.run_command


def _run_command_nobirsim(argv, **kw):
    argv = ["--enable-birsim=false" if a == "--enable-birsim=true" else a
            for a in argv]
    return _orig_run_command(argv, **kw)


bass_utils.run_command = _run_command_nobirsim


# --- custom DVE op: inclusive scan with state = |state - x| ------------------
def _register_scan_op() -> DveOp:
    name = "ABS_DIFF_SCAN_ANT"
    if name in dve_ops_mod._SUB_OPCODE_FOR_NAME:
        return next(o for o in dve_ops_mod.OPS if o.name == name)
    spec = Spec(body=SpecScan(DAlu.ABSOLUTE_DIFF, Src0, init=Zero))
    row = max(dve_ops_mod._SUB_OPCODE_FOR_NAME.values()) + 1
    assert row < 0x20
    dve_ops_mod._SUB_OPCODE_FOR_NAME[name] = row
    sha = DveOpSpec(name=name, opcode=row, uops=dve_lower(spec, ver="v3"),
                    rd1_en=_has_src1(spec)).sha("v3")
    op = DveOp(name, spec, subdim=False, uops_sha={"v3": sha})
    dve_ops_mod.OPS.append(op)
    dve_ops_mod.CUSTOM_DVE_SPECS[name] = spec
    return op


_SCAN_OP = _register_scan_op()
_BUILD_CACHE: dict = {}


def _build(repeat: int = 1):
    """Build + compile the per-core Bass module (same NEFF on all 8 cores)."""
    cache_key = (repeat, GEMM_DTYPE)
    if cache_key in _BUILD_CACHE:
        return _BUILD_CACHE[cache_key]
    GD = BF16 if GEMM_DTYPE == "bf16" else F32R
    nc = bacc.Bacc("TRN2", target_bir_lowering=False, debug=False)
    X = nc.dram_tensor("X", [I, BC, S], GD, kind="ExternalInput").ap()
    WT = nc.dram_tensor("WT", [I, H], GD, kind="ExternalInput").ap()
    WHO = nc.dram_tensor("WHO", [H, O], F32, kind="ExternalInput").ap()
    BIA = nc.dram_tensor("BIA", [128, O // 128], F32, kind="ExternalInput").ap()
    OUT = nc.dram_tensor("out", [O, BC], F32, kind="ExternalOutput").ap()

    with tile.TileContext(nc) as tc, ExitStack() as ctx:
        const = ctx.enter_context(tc.tile_pool(name="const", bufs=1))
        xp = ctx.enter_context(tc.tile_pool(name="xp", bufs=3))
        gp = ctx.enter_context(tc.tile_pool(name="gp", bufs=2))
        pp = ctx.enter_context(tc.tile_pool(name="pp", bufs=2, space="PSUM"))
        outp = ctx.enter_context(tc.tile_pool(name="outp", bufs=1))

        wt0 = const.tile([128, H], GD, tag="wt0")
        nc.sync.dma_start(wt0[:], WT[0:128, :])
        wt1 = const.tile([128, H], GD, tag="wt1")
        nc.sync.dma_start(wt1[:], WT[128:256, :])
        who = const.tile([128, HB * O], F32, tag="who")
        nc.sync.dma_start(
            who[:].rearrange("p (g o) -> p g o", g=HB, o=O),
            WHO.rearrange("(g p) o -> p g o", p=128),
        )
        bia = const.tile([128, O // 128], F32, tag="bia")
        nc.sync.dma_start(bia[:], BIA[:])

        for _rep in range(repeat):
            h_all = outp.tile([128, HB * BC], F32, tag="h_all")
            for b in range(BC):
                x0 = xp.tile([128, S], GD, tag="x0")
                nc.sync.dma_start(x0[:], X[0:128, b, :])
                x1 = xp.tile([128, S], GD, tag="x1")
                nc.sync.dma_start(x1[:], X[128:256, b, :])

                for hb in range(HB):
                    # qn for (b, hb): [128 h_sub, 2048 t] accumulated in PSUM
                    pt = pp.tile([128, S], F32, tag="ps")
                    for tci in range(NTC):
                        sl = slice(tci * 512, tci * 512 + 512)
                        nc.tensor.matmul(pt[:, sl],
                                         wt0[:, hb * 128:(hb + 1) * 128],
                                         x0[:, sl], start=True, stop=False)
                        nc.tensor.matmul(pt[:, sl],
                                         wt1[:, hb * 128:(hb + 1) * 128],
                                         x1[:, sl], start=False, stop=True)
                    # scan straight out of PSUM: g_t = |g_{t-1} - qn_t| along t
                    g = gp.tile([128, S], F32, tag="g")
                    nc.vector._custom_dve(_SCAN_OP, out=g[:], in0=pt[:])
                    # keep only g_S
                    nc.scalar.copy(h_all[:, hb * BC + b: hb * BC + b + 1],
                                   g[:, S - 1:S])

            # output projection: out[o, b] = sum_h WHO[h, o] * h_all[h, b] + bias
            for ob in range(O // 128):
                po = pp.tile([128, BC], F32, tag="ps")
                for hb in range(HB):
                    lhs = who[:, hb * O + ob * 128: hb * O + ob * 128 + 128]
                    rhs = h_all[:, hb * BC:(hb + 1) * BC]
                    nc.tensor.matmul(po[:], lhs, rhs,
                                     start=(hb == 0), stop=(hb == HB - 1))
                ot = outp.tile([128, BC], F32, tag=f"ot{ob}")
                nc.scalar.activation(ot[:], po[:], Act.Identity,
                                     bias=bia[:, ob:ob + 1], scale=1.0)
                nc.sync.dma_start(OUT[ob * 128:(ob + 1) * 128, :], ot[:])

    nc.compile()
    _BUILD_CACHE[cache_key] = nc
    return nc


def _prep_inputs(X, W_ih, hh, W_ho, b_ho):
    X = np.asarray(X, dtype=np.float32)
    W_ih = np.asarray(W_ih, dtype=np.float32)
    hh = np.asarray(hh, dtype=np.float32).reshape(-1)
    W_ho = np.asarray(W_ho, dtype=np.float32)
    b_ho = np.asarray(b_ho, dtype=np.float32).reshape(-1)
    c = float(hh[0])
    assert np.allclose(hh, c), "kernel assumes uniform hh (setup_inputs gives 0.99)"
    assert 0.0 < c, "scan rescaling requires positive hh"

    # fold -c^{-(t+1)} into X; fold c^S into W_ho
    tscale = (-np.power(np.float64(c), -(np.arange(S, dtype=np.float64) + 1.0))
              ).astype(np.float32)
    Xs = X * tscale[:, None, None]

    if GEMM_DTYPE == "bf16":
        import ml_dtypes
        gnp = ml_dtypes.bfloat16
        Xs = Xs.astype(gnp)
        WT_h = np.ascontiguousarray(W_ih.T.astype(gnp))                   # [I, H]
    else:
        WT_h = np.ascontiguousarray(W_ih.T)                               # [I, H]
    WHO_h = np.ascontiguousarray((W_ho * np.float32(c ** S)).T)           # [H, O]
    BIA_h = np.ascontiguousarray(b_ho.reshape(O // 128, 128).T)           # [128, 2]

    in_maps = []
    for k in range(NCORES):
        xc = np.ascontiguousarray(
            Xs[:, k * BC:(k + 1) * BC, :].transpose(2, 1, 0))             # [I, BC, S]
        in_maps.append(dict(X=xc, WT=WT_h, WHO=WHO_h, BIA=BIA_h))
    return in_maps


def _run(nc, in_maps):
    res = bass_utils.run_bass_kernel_spmd(nc, in_maps, core_ids=list(range(NCORES)))
    return np.concatenate(
        [res.results[k]["out"].T for k in range(NCORES)], axis=0)        # [B, O]


def kernel(X, W_ih, hh, W_ho, b_ho):
    in_maps = _prep_inputs(X, W_ih, hh, W_ho, b_ho)
    nc = _build(repeat=1)
    return _run(nc, in_maps).astype(np.float32)


# revision 10
# speedup vs baseline: 756.0765x; 1.8198x over previous
"""Trainium2 Bass kernel for nn_AbsDiagNet: out = scan(|p_t + c*h|) @ W_ho.T + b_ho.

Algorithm
---------
reference:  pre = einsum('sbi,hi->sbh', X, W_ih)          # big GEMM
            h_{t+1} = |pre[t] + c * h_t|, h_0 = 0         # serial scan, c=hh[0]
            out = h_S @ W_ho.T + b_ho

Key transform: abs is positively homogeneous, so with g_t := h_t * c^{-t}:
            g_{t+1} = | g_t + pre[t] * c^{-(t+1)} |
The per-step multiply disappears. We fold -c^{-(t+1)} into X on the host, so
the device GEMM directly produces  qn[t] = -pre[t]*c^{-(t+1)},  and the scan is
            g_{t+1} = | g_t - qn[t] |   (ABSOLUTE_DIFF)
which is a single-ALU-op recurrence that a custom DVE op runs at
1 element/lane/cycle along the free axis. h_S = c^S * g_S is folded into W_ho.

Suffix truncation: the recurrence is exponentially forgetting -- in h-domain
|dh_S / dh_t| = c^(S-t), so starting the scan at t = S-L with h=0 perturbs the
output by at most ~1.2*c^L (relative, conservative bound incl. the output
projection). L is chosen adaptively from the runtime c so this bound is
<= 1e-4 (for c=0.99: L=1024, measured truncation error 7.5e-6 -- 300x below
the bf16 GEMM noise). Halves GEMM/scan/DMA work.

Sharding: data-parallel over batch B=128 -> 16 per core x 8 cores. X is
host-transposed per core to [I, Bc, S] so the GEMM needs no on-device
transposes (contraction dim I lands on partitions). GEMM inputs are bf16
(see GEMM_DTYPE); PSUM accumulation and the scan are fp32.
"""
import os
import numpy as np
from contextlib import ExitStack

import concourse.bass as bass
import concourse.tile as tile
from concourse import bacc, mybir
import concourse.bass_utils as bass_utils
import concourse.dve_ops as dve_ops_mod
from concourse.dve_ops import DveOp
from concourse.dve_spec import (
    Spec, Scan as SpecScan, AluOp as DAlu, Src0, Zero, lower as dve_lower,
    _has_src1,
)
from concourse.dve_uop import DveOpSpec

F32 = mybir.dt.float32
F32R = mybir.dt.float32r
BF16 = mybir.dt.bfloat16
Act = mybir.ActivationFunctionType

# GEMM input dtype: "bf16" (end-to-end relerr ~2.3e-3, ~296us/core) or
# "f32r" (~12 mantissa bits, relerr ~1.4e-4, ~330us/core). Both are far inside
# the 2e-2 gate; the wall is the DVE scan floor (~292us cost-model) either
# way, but bf16's fast FWL weight loads keep the 1024 matmuls fully hidden.
GEMM_DTYPE = "bf16"

S, B, I, H, O = 2048, 128, 256, 1024, 256
NCORES = 8
BC = B // NCORES          # 16 batches per core
HB = H // 128             # 8 h-blocks
TRUNC_TOL = 1e-4          # conservative relative-error budget for truncation


def _suffix_len(c: float) -> int:
    """Scan suffix length L: smallest 512-multiple with 1.2*c^L <= TRUNC_TOL."""
    import math
    if c >= 1.0 or c <= 0.0:
        return S
    L = math.log(1.2 / TRUNC_TOL) / -math.log(c)
    return min(S, max(512, int(math.ceil(L / 512.0)) * 512))

# --- disable walrus birsim (verification-only; big compile-time cost) -------
_orig_run_command = bass_utils.run_command


def _run_command_nobirsim(argv, **kw):
    argv = ["--enable-birsim=false" if a == "--enable-birsim=true" else a
            for a in argv]
    return _orig_run_command(argv, **kw)


bass_utils.run_command = _run_command_nobirsim


# --- custom DVE op: inclusive scan with state = |state - x| ------------------
def _register_scan_op() -> DveOp:
    name = "ABS_DIFF_SCAN_ANT"
    if name in dve_ops_mod._SUB_OPCODE_FOR_NAME:
        return next(o for o in dve_ops_mod.OPS if o.name == name)
    spec = Spec(body=SpecScan(DAlu.ABSOLUTE_DIFF, Src0, init=Zero))
    row = max(dve_ops_mod._SUB_OPCODE_FOR_NAME.values()) + 1
    assert row < 0x20
    dve_ops_mod._SUB_OPCODE_FOR_NAME[name] = row
    sha = DveOpSpec(name=name, opcode=row, uops=dve_lower(spec, ver="v3"),
                    rd1_en=_has_src1(spec)).sha("v3")
    op = DveOp(name, spec, subdim=False, uops_sha={"v3": sha})
    dve_ops_mod.OPS.append(op)
    dve_ops_mod.CUSTOM_DVE_SPECS[name] = spec
    return op


_SCAN_OP = _register_scan_op()
_BUILD_CACHE: dict = {}


def _build(repeat: int = 1, L: int = S):
    """Build + compile the per-core Bass module (same NEFF on all 8 cores)."""
    cache_key = (repeat, GEMM_DTYPE, L)
    if cache_key in _BUILD_CACHE:
        return _BUILD_CACHE[cache_key]
    NTC = L // 512            # matmul t-chunks of 512
    GD = BF16 if GEMM_DTYPE == "bf16" else F32R
    nc = bacc.Bacc("TRN2", target_bir_lowering=False, debug=False)
    X = nc.dram_tensor("X", [I, BC, L], GD, kind="ExternalInput").ap()
    WT = nc.dram_tensor("WT", [I, H], GD, kind="ExternalInput").ap()
    WHO = nc.dram_tensor("WHO", [H, O], F32, kind="ExternalInput").ap()
    BIA = nc.dram_tensor("BIA", [128, O // 128], F32, kind="ExternalInput").ap()
    OUT = nc.dram_tensor("out", [O, BC], F32, kind="ExternalOutput").ap()

    with tile.TileContext(nc) as tc, ExitStack() as ctx:
        const = ctx.enter_context(tc.tile_pool(name="const", bufs=1))
        xp = ctx.enter_context(tc.tile_pool(name="xp", bufs=3))
        gp = ctx.enter_context(tc.tile_pool(name="gp", bufs=2))
        pp = ctx.enter_context(tc.tile_pool(name="pp", bufs=2, space="PSUM"))
        outp = ctx.enter_context(tc.tile_pool(name="outp", bufs=1))

        wt0 = const.tile([128, H], GD, tag="wt0")
        nc.sync.dma_start(wt0[:], WT[0:128, :])
        wt1 = const.tile([128, H], GD, tag="wt1")
        nc.sync.dma_start(wt1[:], WT[128:256, :])
        who = const.tile([128, HB * O], F32, tag="who")
        nc.sync.dma_start(
            who[:].rearrange("p (g o) -> p g o", g=HB, o=O),
            WHO.rearrange("(g p) o -> p g o", p=128),
        )
        bia = const.tile([128, O // 128], F32, tag="bia")
        nc.sync.dma_start(bia[:], BIA[:])

        for _rep in range(repeat):
            h_all = outp.tile([128, HB * BC], F32, tag="h_all")
            for b in range(BC):
                x0 = xp.tile([128, L], GD, tag="x0")
                nc.sync.dma_start(x0[:], X[0:128, b, :])
                x1 = xp.tile([128, L], GD, tag="x1")
                nc.sync.dma_start(x1[:], X[128:256, b, :])

                for hb in range(HB):
                    # qn for (b, hb): [128 h_sub, 2048 t] accumulated in PSUM
                    pt = pp.tile([128, L], F32, tag="ps")
                    for tci in range(NTC):
                        sl = slice(tci * 512, tci * 512 + 512)
                        nc.tensor.matmul(pt[:, sl],
                                         wt0[:, hb * 128:(hb + 1) * 128],
                                         x0[:, sl], start=True, stop=False)
                        nc.tensor.matmul(pt[:, sl],
                                         wt1[:, hb * 128:(hb + 1) * 128],
                                         x1[:, sl], start=False, stop=True)
                    # scan straight out of PSUM: g_t = |g_{t-1} - qn_t| along t
                    g = gp.tile([128, L], F32, tag="g")
                    nc.vector._custom_dve(_SCAN_OP, out=g[:], in0=pt[:])
                    # keep only g_L
                    nc.scalar.copy(h_all[:, hb * BC + b: hb * BC + b + 1],
                                   g[:, L - 1:L])

            # output projection: out[o, b] = sum_h WHO[h, o] * h_all[h, b] + bias
            for ob in range(O // 128):
                po = pp.tile([128, BC], F32, tag="ps")
                for hb in range(HB):
                    lhs = who[:, hb * O + ob * 128: hb * O + ob * 128 + 128]
                    rhs = h_all[:, hb * BC:(hb + 1) * BC]
                    nc.tensor.matmul(po[:], lhs, rhs,
                                     start=(hb == 0), stop=(hb == HB - 1))
                ot = outp.tile([128, BC], F32, tag=f"ot{ob}")
                nc.scalar.activation(ot[:], po[:], Act.Identity,
                                     bias=bia[:, ob:ob + 1], scale=1.0)
                nc.sync.dma_start(OUT[ob * 128:(ob + 1) * 128, :], ot[:])

    nc.compile()
    _BUILD_CACHE[cache_key] = nc
    return nc


def _prep_inputs(X, W_ih, hh, W_ho, b_ho):
    X = np.asarray(X, dtype=np.float32)
    W_ih = np.asarray(W_ih, dtype=np.float32)
    hh = np.asarray(hh, dtype=np.float32).reshape(-1)
    W_ho = np.asarray(W_ho, dtype=np.float32)
    b_ho = np.asarray(b_ho, dtype=np.float32).reshape(-1)
    c = float(hh[0])
    assert np.allclose(hh, c), "kernel assumes uniform hh (setup_inputs gives 0.99)"
    assert 0.0 < c, "scan rescaling requires positive hh"

    # truncate to the last L steps (see module docstring), restart h=0 there;
    # fold -c^{-(k+1)} into X; fold c^L into W_ho
    L = _suffix_len(c)
    X = X[S - L:]
    tscale = (-np.power(np.float64(c), -(np.arange(L, dtype=np.float64) + 1.0))
              ).astype(np.float32)
    Xs = X * tscale[:, None, None]

    if GEMM_DTYPE == "bf16":
        import ml_dtypes
        gnp = ml_dtypes.bfloat16
        Xs = Xs.astype(gnp)
        WT_h = np.ascontiguousarray(W_ih.T.astype(gnp))                   # [I, H]
    else:
        WT_h = np.ascontiguousarray(W_ih.T)                               # [I, H]
    WHO_h = np.ascontiguousarray((W_ho * np.float32(c ** L)).T)           # [H, O]
    BIA_h = np.ascontiguousarray(b_ho.reshape(O // 128, 128).T)           # [128, 2]

    in_maps = []
    for k in range(NCORES):
        xc = np.ascontiguousarray(
            Xs[:, k * BC:(k + 1) * BC, :].transpose(2, 1, 0))             # [I, BC, L]
        in_maps.append(dict(X=xc, WT=WT_h, WHO=WHO_h, BIA=BIA_h))
    return in_maps, L


def _run(nc, in_maps):
    res = bass_utils.run_bass_kernel_spmd(nc, in_maps, core_ids=list(range(NCORES)))
    return np.concatenate(
        [res.results[k]["out"].T for k in range(NCORES)], axis=0)        # [B, O]


def kernel(X, W_ih, hh, W_ho, b_ho):
    in_maps, L = _prep_inputs(X, W_ih, hh, W_ho, b_ho)
    nc = _build(repeat=1, L=L)
    return _run(nc, in_maps).astype(np.float32)


# revision 11
# speedup vs baseline: 961.1832x; 1.2713x over previous
"""Trainium2 Bass kernel for nn_AbsDiagNet: out = scan(|p_t + c*h|) @ W_ho.T + b_ho.

Algorithm
---------
reference:  pre = einsum('sbi,hi->sbh', X, W_ih)          # big GEMM
            h_{t+1} = |pre[t] + c * h_t|, h_0 = 0         # serial scan, c=hh[0]
            out = h_S @ W_ho.T + b_ho

Key transform: abs is positively homogeneous, so with g_t := h_t * c^{-t}:
            g_{t+1} = | g_t + pre[t] * c^{-(t+1)} |
The per-step multiply disappears. We fold -c^{-(t+1)} into X on the host, so
the device GEMM directly produces  qn[t] = -pre[t]*c^{-(t+1)},  and the scan is
            g_{t+1} = | g_t - qn[t] |   (ABSOLUTE_DIFF)
which is a single-ALU-op recurrence that a custom DVE op runs at
1 element/lane/cycle along the free axis. h_S = c^S * g_S is folded into W_ho.

Suffix truncation: the recurrence is exponentially forgetting -- in h-domain
|dh_S / dh_t| = c^(S-t), so starting the scan at t = S-L with h=0 perturbs the
output by at most ~1.2*c^L (relative, conservative bound incl. the output
projection). L is chosen adaptively from the runtime c so this bound is
<= 1e-4 (for c=0.99: L=1024, measured truncation error 7.5e-6 -- 300x below
the bf16 GEMM noise). Halves GEMM/scan/DMA work.

Sharding: data-parallel over batch B=128 -> 16 per core x 8 cores. X is
host-transposed per core to [I, Bc, S] so the GEMM needs no on-device
transposes (contraction dim I lands on partitions). GEMM inputs are bf16
(see GEMM_DTYPE); PSUM accumulation and the scan are fp32.
"""
import os
import numpy as np
from contextlib import ExitStack

import concourse.bass as bass
import concourse.tile as tile
from concourse import bacc, mybir
import concourse.bass_utils as bass_utils
import concourse.dve_ops as dve_ops_mod
from concourse.dve_ops import DveOp
from concourse.dve_spec import (
    Spec, Scan as SpecScan, AluOp as DAlu, Src0, Zero, lower as dve_lower,
    _has_src1,
)
from concourse.dve_uop import DveOpSpec

F32 = mybir.dt.float32
F32R = mybir.dt.float32r
BF16 = mybir.dt.bfloat16
Act = mybir.ActivationFunctionType

# GEMM input dtype: "bf16" (end-to-end relerr ~2.3e-3, ~296us/core) or
# "f32r" (~12 mantissa bits, relerr ~1.4e-4, ~330us/core). Both are far inside
# the 2e-2 gate; the wall is the DVE scan floor (~292us cost-model) either
# way, but bf16's fast FWL weight loads keep the 1024 matmuls fully hidden.
GEMM_DTYPE = "bf16"

S, B, I, H, O = 2048, 128, 256, 1024, 256
NCORES = 8
BC = B // NCORES          # 16 batches per core
HB = H // 128             # 8 h-blocks
TRUNC_TOL = 6e-4          # conservative relative-error budget for truncation


def _suffix_len(c: float) -> int:
    """Scan suffix length L: smallest 256-multiple with 1.2*c^L <= TRUNC_TOL.

    The bound is ~6x conservative vs measured (sign cancellation in the
    output projection); at c=0.99 it gives L=768 where the measured
    truncation error is 8.3e-5 -- 30x below the bf16 GEMM noise."""
    import math
    if c >= 1.0 or c <= 0.0:
        return S
    L = math.log(1.2 / TRUNC_TOL) / -math.log(c)
    return min(S, max(512, int(math.ceil(L / 256.0)) * 256))

# --- disable walrus birsim (verification-only; big compile-time cost) -------
_orig_run_command = bass_utils.run_command


def _run_command_nobirsim(argv, **kw):
    argv = ["--enable-birsim=false" if a == "--enable-birsim=true" else a
            for a in argv]
    return _orig_run_command(argv, **kw)


bass_utils.run_command = _run_command_nobirsim


# --- custom DVE op: inclusive scan with state = |state - x| ------------------
def _register_scan_op() -> DveOp:
    name = "ABS_DIFF_SCAN_ANT"
    if name in dve_ops_mod._SUB_OPCODE_FOR_NAME:
        return next(o for o in dve_ops_mod.OPS if o.name == name)
    spec = Spec(body=SpecScan(DAlu.ABSOLUTE_DIFF, Src0, init=Zero))
    row = max(dve_ops_mod._SUB_OPCODE_FOR_NAME.values()) + 1
    assert row < 0x20
    dve_ops_mod._SUB_OPCODE_FOR_NAME[name] = row
    sha = DveOpSpec(name=name, opcode=row, uops=dve_lower(spec, ver="v3"),
                    rd1_en=_has_src1(spec)).sha("v3")
    op = DveOp(name, spec, subdim=False, uops_sha={"v3": sha})
    dve_ops_mod.OPS.append(op)
    dve_ops_mod.CUSTOM_DVE_SPECS[name] = spec
    return op


_SCAN_OP = _register_scan_op()
_BUILD_CACHE: dict = {}


def _build(repeat: int = 1, L: int = S):
    """Build + compile the per-core Bass module (same NEFF on all 8 cores)."""
    cache_key = (repeat, GEMM_DTYPE, L)
    if cache_key in _BUILD_CACHE:
        return _BUILD_CACHE[cache_key]
    assert L % 256 == 0
    # matmul t-chunks: 512-wide, with a trailing 256 if L % 512
    chunks = [(i * 512, 512) for i in range(L // 512)]
    if L % 512:
        chunks.append((L - 256, 256))
    GD = BF16 if GEMM_DTYPE == "bf16" else F32R
    nc = bacc.Bacc("TRN2", target_bir_lowering=False, debug=False)
    X = nc.dram_tensor("X", [I, BC, L], GD, kind="ExternalInput").ap()
    WT = nc.dram_tensor("WT", [I, H], GD, kind="ExternalInput").ap()
    WHO = nc.dram_tensor("WHO", [H, O], F32, kind="ExternalInput").ap()
    BIA = nc.dram_tensor("BIA", [128, O // 128], F32, kind="ExternalInput").ap()
    OUT = nc.dram_tensor("out", [O, BC], F32, kind="ExternalOutput").ap()

    with tile.TileContext(nc) as tc, ExitStack() as ctx:
        const = ctx.enter_context(tc.tile_pool(name="const", bufs=1))
        xp = ctx.enter_context(tc.tile_pool(name="xp", bufs=3))
        gp = ctx.enter_context(tc.tile_pool(name="gp", bufs=2))
        pp = ctx.enter_context(tc.tile_pool(name="pp", bufs=2, space="PSUM"))
        outp = ctx.enter_context(tc.tile_pool(name="outp", bufs=1))

        wt0 = const.tile([128, H], GD, tag="wt0")
        nc.sync.dma_start(wt0[:], WT[0:128, :])
        wt1 = const.tile([128, H], GD, tag="wt1")
        nc.sync.dma_start(wt1[:], WT[128:256, :])
        who = const.tile([128, HB * O], F32, tag="who")
        nc.sync.dma_start(
            who[:].rearrange("p (g o) -> p g o", g=HB, o=O),
            WHO.rearrange("(g p) o -> p g o", p=128),
        )
        bia = const.tile([128, O // 128], F32, tag="bia")
        nc.sync.dma_start(bia[:], BIA[:])

        for _rep in range(repeat):
            h_all = outp.tile([128, HB * BC], F32, tag="h_all")
            for b in range(BC):
                x0 = xp.tile([128, L], GD, tag="x0")
                nc.sync.dma_start(x0[:], X[0:128, b, :])
                x1 = xp.tile([128, L], GD, tag="x1")
                nc.sync.dma_start(x1[:], X[128:256, b, :])

                for hb in range(HB):
                    # qn for (b, hb): [128 h_sub, 2048 t] accumulated in PSUM
                    pt = pp.tile([128, L], F32, tag="ps")
                    for off, width in chunks:
                        sl = slice(off, off + width)
                        nc.tensor.matmul(pt[:, sl],
                                         wt0[:, hb * 128:(hb + 1) * 128],
                                         x0[:, sl], start=True, stop=False)
                        nc.tensor.matmul(pt[:, sl],
                                         wt1[:, hb * 128:(hb + 1) * 128],
                                         x1[:, sl], start=False, stop=True)
                    # scan straight out of PSUM: g_t = |g_{t-1} - qn_t| along t
                    g = gp.tile([128, L], F32, tag="g")
                    nc.vector._custom_dve(_SCAN_OP, out=g[:], in0=pt[:])
                    # keep only g_L
                    nc.scalar.copy(h_all[:, hb * BC + b: hb * BC + b + 1],
                                   g[:, L - 1:L])

            # output projection: out[o, b] = sum_h WHO[h, o] * h_all[h, b] + bias
            for ob in range(O // 128):
                po = pp.tile([128, BC], F32, tag="ps")
                for hb in range(HB):
                    lhs = who[:, hb * O + ob * 128: hb * O + ob * 128 + 128]
                    rhs = h_all[:, hb * BC:(hb + 1) * BC]
                    nc.tensor.matmul(po[:], lhs, rhs,
                                     start=(hb == 0), stop=(hb == HB - 1))
                ot = outp.tile([128, BC], F32, tag=f"ot{ob}")
                nc.scalar.activation(ot[:], po[:], Act.Identity,
                                     bias=bia[:, ob:ob + 1], scale=1.0)
                nc.sync.dma_start(OUT[ob * 128:(ob + 1) * 128, :], ot[:])

    nc.compile()
    _BUILD_CACHE[cache_key] = nc
    return nc


def _prep_inputs(X, W_ih, hh, W_ho, b_ho):
    X = np.asarray(X, dtype=np.float32)
    W_ih = np.asarray(W_ih, dtype=np.float32)
    hh = np.asarray(hh, dtype=np.float32).reshape(-1)
    W_ho = np.asarray(W_ho, dtype=np.float32)
    b_ho = np.asarray(b_ho, dtype=np.float32).reshape(-1)
    c = float(hh[0])
    assert np.allclose(hh, c), "kernel assumes uniform hh (setup_inputs gives 0.99)"
    assert 0.0 < c, "scan rescaling requires positive hh"

    # truncate to the last L steps (see module docstring), restart h=0 there;
    # fold -c^{-(k+1)} into X; fold c^L into W_ho
    L = _suffix_len(c)
    X = X[S - L:]
    tscale = (-np.power(np.float64(c), -(np.arange(L, dtype=np.float64) + 1.0))
              ).astype(np.float32)
    Xs = X * tscale[:, None, None]

    if GEMM_DTYPE == "bf16":
        import ml_dtypes
        gnp = ml_dtypes.bfloat16
        Xs = Xs.astype(gnp)
        WT_h = np.ascontiguousarray(W_ih.T.astype(gnp))                   # [I, H]
    else:
        WT_h = np.ascontiguousarray(W_ih.T)                               # [I, H]
    WHO_h = np.ascontiguousarray((W_ho * np.float32(c ** L)).T)           # [H, O]
    BIA_h = np.ascontiguousarray(b_ho.reshape(O // 128, 128).T)           # [128, 2]

    in_maps = []
    for k in range(NCORES):
        xc = np.ascontiguousarray(
            Xs[:, k * BC:(k + 1) * BC, :].transpose(2, 1, 0))             # [I, BC, L]
        in_maps.append(dict(X=xc, WT=WT_h, WHO=WHO_h, BIA=BIA_h))
    return in_maps, L


def _run(nc, in_maps):
    res = bass_utils.run_bass_kernel_spmd(nc, in_maps, core_ids=list(range(NCORES)))
    return np.concatenate(
        [res.results[k]["out"].T for k in range(NCORES)], axis=0)        # [B, O]


def kernel(X, W_ih, hh, W_ho, b_ho):
    in_maps, L = _prep_inputs(X, W_ih, hh, W_ho, b_ho)
    nc = _build(repeat=1, L=L)
    return _run(nc, in_maps).astype(np.float32)


# revision 12
# speedup vs baseline: 1026.8095x; 1.0683x over previous
"""Trainium2 Bass kernel for nn_AbsDiagNet: out = scan(|p_t + c*h|) @ W_ho.T + b_ho.

Algorithm
---------
reference:  pre = einsum('sbi,hi->sbh', X, W_ih)          # big GEMM
            h_{t+1} = |pre[t] + c * h_t|, h_0 = 0         # serial scan, c=hh[0]
            out = h_S @ W_ho.T + b_ho

Key transform: abs is positively homogeneous, so with g_t := h_t * c^{-t}:
            g_{t+1} = | g_t + pre[t] * c^{-(t+1)} |
The per-step multiply disappears. We fold -c^{-(t+1)} into X on the host, so
the device GEMM directly produces  qn[t] = -pre[t]*c^{-(t+1)},  and the scan is
            g_{t+1} = | g_t - qn[t] |   (ABSOLUTE_DIFF)
which is a single-ALU-op recurrence that a custom DVE op runs at
1 element/lane/cycle along the free axis. h_S = c^S * g_S is folded into W_ho.

Suffix truncation: the recurrence is exponentially forgetting -- in h-domain
|dh_S / dh_t| = c^(S-t), so starting the scan at t = S-L with h=0 perturbs the
output by at most ~1.2*c^L (relative, conservative bound incl. the output
projection). L is chosen adaptively from the runtime c so this bound is
<= 1e-4 (for c=0.99: L=1024, measured truncation error 7.5e-6 -- 300x below
the bf16 GEMM noise). Halves GEMM/scan/DMA work.

Sharding: data-parallel over batch B=128 -> 16 per core x 8 cores. X is
host-transposed per core to [I, Bc, S] so the GEMM needs no on-device
transposes (contraction dim I lands on partitions). GEMM inputs are bf16
(see GEMM_DTYPE); PSUM accumulation and the scan are fp32.
"""
import os
import numpy as np
from contextlib import ExitStack

import concourse.bass as bass
import concourse.tile as tile
from concourse import bacc, mybir
import concourse.bass_utils as bass_utils
import concourse.dve_ops as dve_ops_mod
from concourse.dve_ops import DveOp
from concourse.dve_spec import (
    Spec, Scan as SpecScan, AluOp as DAlu, Src0, Zero, lower as dve_lower,
    _has_src1,
)
from concourse.dve_uop import DveOpSpec

F32 = mybir.dt.float32
F32R = mybir.dt.float32r
BF16 = mybir.dt.bfloat16
Act = mybir.ActivationFunctionType

# GEMM input dtype: "bf16" (end-to-end relerr ~2.3e-3, ~296us/core) or
# "f32r" (~12 mantissa bits, relerr ~1.4e-4, ~330us/core). Both are far inside
# the 2e-2 gate; the wall is the DVE scan floor (~292us cost-model) either
# way, but bf16's fast FWL weight loads keep the 1024 matmuls fully hidden.
GEMM_DTYPE = "bf16"

S, B, I, H, O = 2048, 128, 256, 1024, 256
NCORES = 8
BC = B // NCORES          # 16 batches per core
HB = H // 128             # 8 h-blocks
TRUNC_TOL = 6e-4          # conservative relative-error budget for truncation


def _suffix_len(c: float) -> int:
    """Scan suffix length L: smallest 256-multiple with 1.2*c^L <= TRUNC_TOL.

    The bound is ~6x conservative vs measured (sign cancellation in the
    output projection); at c=0.99 it gives L=768 where the measured
    truncation error is 8.3e-5 -- 30x below the bf16 GEMM noise."""
    import math
    if c >= 1.0 or c <= 0.0:
        return S
    L = math.log(1.2 / TRUNC_TOL) / -math.log(c)
    return min(S, max(512, int(math.ceil(L / 256.0)) * 256))

# --- disable walrus birsim (verification-only; big compile-time cost) -------
_orig_run_command = bass_utils.run_command


def _run_command_nobirsim(argv, **kw):
    argv = ["--enable-birsim=false" if a == "--enable-birsim=true" else a
            for a in argv]
    return _orig_run_command(argv, **kw)


bass_utils.run_command = _run_command_nobirsim


# --- custom DVE op: inclusive scan with state = |state - x| ------------------
def _register_scan_op() -> DveOp:
    name = "ABS_DIFF_SCAN_ANT"
    if name in dve_ops_mod._SUB_OPCODE_FOR_NAME:
        return next(o for o in dve_ops_mod.OPS if o.name == name)
    spec = Spec(body=SpecScan(DAlu.ABSOLUTE_DIFF, Src0, init=Zero))
    row = max(dve_ops_mod._SUB_OPCODE_FOR_NAME.values()) + 1
    assert row < 0x20
    dve_ops_mod._SUB_OPCODE_FOR_NAME[name] = row
    sha = DveOpSpec(name=name, opcode=row, uops=dve_lower(spec, ver="v3"),
                    rd1_en=_has_src1(spec)).sha("v3")
    op = DveOp(name, spec, subdim=False, uops_sha={"v3": sha})
    dve_ops_mod.OPS.append(op)
    dve_ops_mod.CUSTOM_DVE_SPECS[name] = spec
    return op


_SCAN_OP = _register_scan_op()


def _register_scan2_op() -> DveOp:
    """Paired-chain variant: in0/out are [128, 2, L] (2 chains per lane per
    instruction). A third uop re-seeds the scan state to zero when the
    innermost free dim wraps (SUB_DIM_DONE), so the two chains are
    independent. Halves the per-instruction overhead of the scan phase."""
    name = "ABS_DIFF_SCAN2_ANT"
    if name in dve_ops_mod._SUB_OPCODE_FOR_NAME:
        return next(o for o in dve_ops_mod.OPS if o.name == name)
    import copy
    from concourse.dve_uop import Trigger as UTrig
    spec = Spec(body=SpecScan(DAlu.ABSOLUTE_DIFF, Src0, init=Zero))
    seed, steady = dve_lower(spec, ver="v3")
    steady = copy.deepcopy(steady)
    reseed = copy.deepcopy(seed)          # COUNT(1) -> uop 1, no src consumed
    steady.trigger = (UTrig.SRC_TENSOR_DONE, UTrig.SUB_DIM_DONE, UTrig.NONE)
    steady.next_uop = (0, 2, 0)           # end -> IDLE; chain boundary -> reseed
    row = max(dve_ops_mod._SUB_OPCODE_FOR_NAME.values()) + 1
    assert row < 0x20
    dve_ops_mod._SUB_OPCODE_FOR_NAME[name] = row
    spec_obj = DveOpSpec(name=name, opcode=row, uops=[seed, steady, reseed],
                         rd1_en=_has_src1(spec))

    class _HandDveOp(DveOp):
        def compile(self, ver):
            assert ver == "v3", "hand-built uops pinned to v3/TRN2"
            return spec_obj

    op = _HandDveOp(name, spec, subdim=True,
                    uops_sha={"v3": spec_obj.sha("v3")})
    dve_ops_mod.OPS.append(op)
    dve_ops_mod.CUSTOM_DVE_SPECS[name] = spec
    return op


_SCAN2_OP = _register_scan2_op()
SCAN_PAIR = True          # 2 chains per scan instruction (halves DVE overhead)
_BUILD_CACHE: dict = {}


def _build(repeat: int = 1, L: int = S):
    """Build + compile the per-core Bass module (same NEFF on all 8 cores)."""
    cache_key = (repeat, GEMM_DTYPE, L, SCAN_PAIR)
    if cache_key in _BUILD_CACHE:
        return _BUILD_CACHE[cache_key]
    assert L % 256 == 0
    # matmul t-chunks: 512-wide, with a trailing 256 if L % 512
    chunks = [(i * 512, 512) for i in range(L // 512)]
    if L % 512:
        chunks.append((L - 256, 256))
    GD = BF16 if GEMM_DTYPE == "bf16" else F32R
    nc = bacc.Bacc("TRN2", target_bir_lowering=False, debug=False)
    X = nc.dram_tensor("X", [I, BC, L], GD, kind="ExternalInput").ap()
    WT = nc.dram_tensor("WT", [I, H], GD, kind="ExternalInput").ap()
    WHO = nc.dram_tensor("WHO", [H, O], F32, kind="ExternalInput").ap()
    BIA = nc.dram_tensor("BIA", [128, O // 128], F32, kind="ExternalInput").ap()
    OUT = nc.dram_tensor("out", [O, BC], F32, kind="ExternalOutput").ap()

    with tile.TileContext(nc) as tc, ExitStack() as ctx:
        const = ctx.enter_context(tc.tile_pool(name="const", bufs=1))
        xp = ctx.enter_context(tc.tile_pool(name="xp", bufs=3))
        gp = ctx.enter_context(tc.tile_pool(name="gp", bufs=2))
        pp = ctx.enter_context(tc.tile_pool(name="pp", bufs=2, space="PSUM"))
        outp = ctx.enter_context(tc.tile_pool(name="outp", bufs=1))

        wt0 = const.tile([128, H], GD, tag="wt0")
        nc.sync.dma_start(wt0[:], WT[0:128, :])
        wt1 = const.tile([128, H], GD, tag="wt1")
        nc.sync.dma_start(wt1[:], WT[128:256, :])
        who = const.tile([128, HB * O], F32, tag="who")
        nc.sync.dma_start(
            who[:].rearrange("p (g o) -> p g o", g=HB, o=O),
            WHO.rearrange("(g p) o -> p g o", p=128),
        )
        bia = const.tile([128, O // 128], F32, tag="bia")
        nc.sync.dma_start(bia[:], BIA[:])

        for _rep in range(repeat):
            h_all = outp.tile([128, HB * BC], F32, tag="h_all")
            for b in range(BC):
                x0 = xp.tile([128, L], GD, tag="x0")
                nc.sync.dma_start(x0[:], X[0:128, b, :])
                x1 = xp.tile([128, L], GD, tag="x1")
                nc.sync.dma_start(x1[:], X[128:256, b, :])

                if SCAN_PAIR:
                    # chains padded to CP elements in PSUM for bank alignment
                    CP = -(-L // 1024) * 1024
                    hv = h_all[:].rearrange("p (g b) -> p g b", g=HB, b=BC)
                    for hp in range(HB // 2):
                        pt = pp.tile([128, 2 * CP], F32, tag="ps")
                        for j in range(2):
                            hb = 2 * hp + j
                            for off, width in chunks:
                                sl_x = slice(off, off + width)
                                sl_p = slice(j * CP + off, j * CP + off + width)
                                nc.tensor.matmul(
                                    pt[:, sl_p], wt0[:, hb * 128:(hb + 1) * 128],
                                    x0[:, sl_x], start=True, stop=False)
                                nc.tensor.matmul(
                                    pt[:, sl_p], wt1[:, hb * 128:(hb + 1) * 128],
                                    x1[:, sl_x], start=False, stop=True)
                        g = gp.tile([128, 2 * L], F32, tag="g")
                        in_v = pt[:].rearrange("p (c t) -> p c t",
                                               c=2, t=CP)[:, :, 0:L]
                        out_v = g[:].rearrange("p (c t) -> p c t", c=2, t=L)
                        nc.vector._custom_dve(_SCAN2_OP, out=out_v, in0=in_v)
                        nc.scalar.copy(hv[:, 2 * hp:2 * hp + 2, b],
                                       out_v[:, :, L - 1])
                else:
                    for hb in range(HB):
                        # qn for (b, hb): [128 h_sub, L t] accumulated in PSUM
                        pt = pp.tile([128, L], F32, tag="ps")
                        for off, width in chunks:
                            sl = slice(off, off + width)
                            nc.tensor.matmul(pt[:, sl],
                                             wt0[:, hb * 128:(hb + 1) * 128],
                                             x0[:, sl], start=True, stop=False)
                            nc.tensor.matmul(pt[:, sl],
                                             wt1[:, hb * 128:(hb + 1) * 128],
                                             x1[:, sl], start=False, stop=True)
                        # scan straight out of PSUM: g_t = |g_{t-1} - qn_t|
                        g = gp.tile([128, L], F32, tag="g")
                        nc.vector._custom_dve(_SCAN_OP, out=g[:], in0=pt[:])
                        # keep only g_L
                        nc.scalar.copy(h_all[:, hb * BC + b: hb * BC + b + 1],
                                       g[:, L - 1:L])

            # output projection: out[o, b] = sum_h WHO[h, o] * h_all[h, b] + bias
            for ob in range(O // 128):
                po = pp.tile([128, BC], F32, tag="ps")
                for hb in range(HB):
                    lhs = who[:, hb * O + ob * 128: hb * O + ob * 128 + 128]
                    rhs = h_all[:, hb * BC:(hb + 1) * BC]
                    nc.tensor.matmul(po[:], lhs, rhs,
                                     start=(hb == 0), stop=(hb == HB - 1))
                ot = outp.tile([128, BC], F32, tag=f"ot{ob}")
                nc.scalar.activation(ot[:], po[:], Act.Identity,
                                     bias=bia[:, ob:ob + 1], scale=1.0)
                nc.sync.dma_start(OUT[ob * 128:(ob + 1) * 128, :], ot[:])

    nc.compile()
    _BUILD_CACHE[cache_key] = nc
    return nc


def _prep_inputs(X, W_ih, hh, W_ho, b_ho):
    X = np.asarray(X, dtype=np.float32)
    W_ih = np.asarray(W_ih, dtype=np.float32)
    hh = np.asarray(hh, dtype=np.float32).reshape(-1)
    W_ho = np.asarray(W_ho, dtype=np.float32)
    b_ho = np.asarray(b_ho, dtype=np.float32).reshape(-1)
    c = float(hh[0])
    assert np.allclose(hh, c), "kernel assumes uniform hh (setup_inputs gives 0.99)"
    assert 0.0 < c, "scan rescaling requires positive hh"

    # truncate to the last L steps (see module docstring), restart h=0 there;
    # fold -c^{-(k+1)} into X; fold c^L into W_ho
    L = _suffix_len(c)
    X = X[S - L:]
    tscale = (-np.power(np.float64(c), -(np.arange(L, dtype=np.float64) + 1.0))
              ).astype(np.float32)
    Xs = X * tscale[:, None, None]

    if GEMM_DTYPE == "bf16":
        import ml_dtypes
        gnp = ml_dtypes.bfloat16
        Xs = Xs.astype(gnp)
        WT_h = np.ascontiguousarray(W_ih.T.astype(gnp))                   # [I, H]
    else:
        WT_h = np.ascontiguousarray(W_ih.T)                               # [I, H]
    WHO_h = np.ascontiguousarray((W_ho * np.float32(c ** L)).T)           # [H, O]
    BIA_h = np.ascontiguousarray(b_ho.reshape(O // 128, 128).T)           # [128, 2]

    in_maps = []
    for k in range(NCORES):
        xc = np.ascontiguousarray(
            Xs[:, k * BC:(k + 1) * BC, :].transpose(2, 1, 0))             # [I, BC, L]
        in_maps.append(dict(X=xc, WT=WT_h, WHO=WHO_h, BIA=BIA_h))
    return in_maps, L


def _run(nc, in_maps):
    res = bass_utils.run_bass_kernel_spmd(nc, in_maps, core_ids=list(range(NCORES)))
    return np.concatenate(
        [res.results[k]["out"].T for k in range(NCORES)], axis=0)        # [B, O]


def kernel(X, W_ih, hh, W_ho, b_ho):
    in_maps, L = _prep_inputs(X, W_ih, hh, W_ho, b_ho)
    nc = _build(repeat=1, L=L)
    return _run(nc, in_maps).astype(np.float32)


# revision 13
# speedup vs baseline: 1093.7208x; 1.0652x over previous
"""Trainium2 Bass kernel for nn_AbsDiagNet: out = scan(|p_t + c*h|) @ W_ho.T + b_ho.

Algorithm
---------
reference:  pre = einsum('sbi,hi->sbh', X, W_ih)          # big GEMM
            h_{t+1} = |pre[t] + c * h_t|, h_0 = 0         # serial scan, c=hh[0]
            out = h_S @ W_ho.T + b_ho

Key transform: abs is positively homogeneous, so with g_t := h_t * c^{-t}:
            g_{t+1} = | g_t + pre[t] * c^{-(t+1)} |
The per-step multiply disappears. We fold -c^{-(t+1)} into X on the host, so
the device GEMM directly produces  qn[t] = -pre[t]*c^{-(t+1)},  and the scan is
            g_{t+1} = | g_t - qn[t] |   (ABSOLUTE_DIFF)
which is a single-ALU-op recurrence that a custom DVE op runs at
1 element/lane/cycle along the free axis. h_S = c^S * g_S is folded into W_ho.

Suffix truncation: the recurrence is exponentially forgetting -- in h-domain
|dh_S / dh_t| = c^(S-t), so starting the scan at t = S-L with h=0 perturbs the
output by at most ~1.2*c^L (relative, conservative bound incl. the output
projection). L is chosen adaptively from the runtime c so this bound is
<= 1e-4 (for c=0.99: L=1024, measured truncation error 7.5e-6 -- 300x below
the bf16 GEMM noise). Halves GEMM/scan/DMA work.

Sharding: data-parallel over batch B=128 -> 16 per core x 8 cores. X is
host-transposed per core to [I, Bc, S] so the GEMM needs no on-device
transposes (contraction dim I lands on partitions). GEMM inputs are bf16
(see GEMM_DTYPE); PSUM accumulation and the scan are fp32.
"""
import os
import numpy as np
from contextlib import ExitStack

import concourse.bass as bass
import concourse.tile as tile
from concourse import bacc, mybir
import concourse.bass_utils as bass_utils
import concourse.dve_ops as dve_ops_mod
from concourse.dve_ops import DveOp
from concourse.dve_spec import (
    Spec, Scan as SpecScan, AluOp as DAlu, Src0, Zero, lower as dve_lower,
    _has_src1,
)
from concourse.dve_uop import DveOpSpec

F32 = mybir.dt.float32
F32R = mybir.dt.float32r
BF16 = mybir.dt.bfloat16
Act = mybir.ActivationFunctionType

# GEMM input dtype: "bf16" (end-to-end relerr ~2.3e-3, ~296us/core) or
# "f32r" (~12 mantissa bits, relerr ~1.4e-4, ~330us/core). Both are far inside
# the 2e-2 gate; the wall is the DVE scan floor (~292us cost-model) either
# way, but bf16's fast FWL weight loads keep the 1024 matmuls fully hidden.
GEMM_DTYPE = "bf16"

S, B, I, H, O = 2048, 128, 256, 1024, 256
NCORES = 8
BC = B // NCORES          # 16 batches per core
HB = H // 128             # 8 h-blocks
TRUNC_TOL = 6e-4          # conservative relative-error budget for truncation


def _suffix_len(c: float) -> int:
    """Scan suffix length L: smallest 256-multiple with 1.2*c^L <= TRUNC_TOL.

    The bound is ~6x conservative vs measured (sign cancellation in the
    output projection); at c=0.99 it gives L=768 where the measured
    truncation error is 8.3e-5 -- 30x below the bf16 GEMM noise."""
    import math
    if c >= 1.0 or c <= 0.0:
        return S
    L = math.log(1.2 / TRUNC_TOL) / -math.log(c)
    return min(S, max(512, int(math.ceil(L / 256.0)) * 256))

# --- disable walrus birsim (verification-only; big compile-time cost) -------
_orig_run_command = bass_utils.run_command


def _run_command_nobirsim(argv, **kw):
    argv = ["--enable-birsim=false" if a == "--enable-birsim=true" else a
            for a in argv]
    return _orig_run_command(argv, **kw)


bass_utils.run_command = _run_command_nobirsim


# --- custom DVE op: inclusive scan with state = |state - x| ------------------
def _register_scan_op() -> DveOp:
    name = "ABS_DIFF_SCAN_ANT"
    if name in dve_ops_mod._SUB_OPCODE_FOR_NAME:
        return next(o for o in dve_ops_mod.OPS if o.name == name)
    spec = Spec(body=SpecScan(DAlu.ABSOLUTE_DIFF, Src0, init=Zero))
    row = max(dve_ops_mod._SUB_OPCODE_FOR_NAME.values()) + 1
    assert row < 0x20
    dve_ops_mod._SUB_OPCODE_FOR_NAME[name] = row
    sha = DveOpSpec(name=name, opcode=row, uops=dve_lower(spec, ver="v3"),
                    rd1_en=_has_src1(spec)).sha("v3")
    op = DveOp(name, spec, subdim=False, uops_sha={"v3": sha})
    dve_ops_mod.OPS.append(op)
    dve_ops_mod.CUSTOM_DVE_SPECS[name] = spec
    return op


_SCAN_OP = _register_scan_op()


def _register_scan2_op() -> DveOp:
    """Paired-chain variant: in0/out are [128, 2, L] (2 chains per lane per
    instruction). A third uop re-seeds the scan state to zero when the
    innermost free dim wraps (SUB_DIM_DONE), so the two chains are
    independent. Halves the per-instruction overhead of the scan phase."""
    name = "ABS_DIFF_SCAN2_ANT"
    if name in dve_ops_mod._SUB_OPCODE_FOR_NAME:
        return next(o for o in dve_ops_mod.OPS if o.name == name)
    import copy
    from concourse.dve_uop import Trigger as UTrig
    spec = Spec(body=SpecScan(DAlu.ABSOLUTE_DIFF, Src0, init=Zero))
    seed, steady = dve_lower(spec, ver="v3")
    steady = copy.deepcopy(steady)
    reseed = copy.deepcopy(seed)          # COUNT(1) -> uop 1, no src consumed
    steady.trigger = (UTrig.SRC_TENSOR_DONE, UTrig.SUB_DIM_DONE, UTrig.NONE)
    steady.next_uop = (0, 2, 0)           # end -> IDLE; chain boundary -> reseed
    row = max(dve_ops_mod._SUB_OPCODE_FOR_NAME.values()) + 1
    assert row < 0x20
    dve_ops_mod._SUB_OPCODE_FOR_NAME[name] = row
    spec_obj = DveOpSpec(name=name, opcode=row, uops=[seed, steady, reseed],
                         rd1_en=_has_src1(spec))

    class _HandDveOp(DveOp):
        def compile(self, ver):
            assert ver == "v3", "hand-built uops pinned to v3/TRN2"
            return spec_obj

    op = _HandDveOp(name, spec, subdim=True,
                    uops_sha={"v3": spec_obj.sha("v3")})
    dve_ops_mod.OPS.append(op)
    dve_ops_mod.CUSTOM_DVE_SPECS[name] = spec
    return op


_SCAN2_OP = _register_scan2_op()
SCAN_PAIR = True          # 2 chains per scan instruction (halves DVE overhead)
_BUILD_CACHE: dict = {}


def _build(repeat: int = 1, L: int = S):
    """Build + compile the per-core Bass module (same NEFF on all 8 cores)."""
    cache_key = (repeat, GEMM_DTYPE, L, SCAN_PAIR)
    if cache_key in _BUILD_CACHE:
        return _BUILD_CACHE[cache_key]
    assert L % 256 == 0
    # matmul t-chunks: 512-wide, with a trailing 256 if L % 512
    chunks = [(i * 512, 512) for i in range(L // 512)]
    if L % 512:
        chunks.append((L - 256, 256))
    GD = BF16 if GEMM_DTYPE == "bf16" else F32R
    nc = bacc.Bacc("TRN2", target_bir_lowering=False, debug=False)
    X = nc.dram_tensor("X", [I, BC, L], GD, kind="ExternalInput").ap()
    WT = nc.dram_tensor("WT", [I, H], GD, kind="ExternalInput").ap()
    WHO = nc.dram_tensor("WHO", [H, O], F32, kind="ExternalInput").ap()
    BIA = nc.dram_tensor("BIA", [128, O // 128], F32, kind="ExternalInput").ap()
    OUT = nc.dram_tensor("out", [O, BC], F32, kind="ExternalOutput").ap()

    with tile.TileContext(nc) as tc, ExitStack() as ctx:
        const = ctx.enter_context(tc.tile_pool(name="const", bufs=1))
        xp = ctx.enter_context(tc.tile_pool(name="xp", bufs=4))
        gp = ctx.enter_context(tc.tile_pool(name="gp", bufs=4))
        pp = ctx.enter_context(tc.tile_pool(name="pp", bufs=2, space="PSUM"))
        outp = ctx.enter_context(tc.tile_pool(name="outp", bufs=2))

        wt0 = const.tile([128, H], GD, tag="wt0")
        nc.sync.dma_start(wt0[:], WT[0:128, :])
        wt1 = const.tile([128, H], GD, tag="wt1")
        nc.sync.dma_start(wt1[:], WT[128:256, :])
        who = const.tile([128, HB * O], F32, tag="who")
        nc.sync.dma_start(
            who[:].rearrange("p (g o) -> p g o", g=HB, o=O),
            WHO.rearrange("(g p) o -> p g o", p=128),
        )
        bia = const.tile([128, O // 128], F32, tag="bia")
        nc.sync.dma_start(bia[:], BIA[:])

        for _rep in range(repeat):
            h_all = outp.tile([128, HB * BC], F32, tag="h_all")
            for b in range(BC):
                x0 = xp.tile([128, L], GD, tag="x0")
                nc.sync.dma_start(x0[:], X[0:128, b, :])
                x1 = xp.tile([128, L], GD, tag="x1")
                nc.sync.dma_start(x1[:], X[128:256, b, :])

                if SCAN_PAIR:
                    # chains padded to CP elements in PSUM for bank alignment
                    CP = -(-L // 1024) * 1024
                    hv = h_all[:].rearrange("p (g b) -> p g b", g=HB, b=BC)
                    for hp in range(HB // 2):
                        pt = pp.tile([128, 2 * CP], F32, tag="ps")
                        for j in range(2):
                            hb = 2 * hp + j
                            for off, width in chunks:
                                sl_x = slice(off, off + width)
                                sl_p = slice(j * CP + off, j * CP + off + width)
                                nc.tensor.matmul(
                                    pt[:, sl_p], wt0[:, hb * 128:(hb + 1) * 128],
                                    x0[:, sl_x], start=True, stop=False)
                                nc.tensor.matmul(
                                    pt[:, sl_p], wt1[:, hb * 128:(hb + 1) * 128],
                                    x1[:, sl_x], start=False, stop=True)
                        g = gp.tile([128, 2 * L], F32, tag="g")
                        in_v = pt[:].rearrange("p (c t) -> p c t",
                                               c=2, t=CP)[:, :, 0:L]
                        out_v = g[:].rearrange("p (c t) -> p c t", c=2, t=L)
                        nc.vector._custom_dve(_SCAN2_OP, out=out_v, in0=in_v)
                        nc.scalar.copy(hv[:, 2 * hp:2 * hp + 2, b],
                                       out_v[:, :, L - 1])
                else:
                    for hb in range(HB):
                        # qn for (b, hb): [128 h_sub, L t] accumulated in PSUM
                        pt = pp.tile([128, L], F32, tag="ps")
                        for off, width in chunks:
                            sl = slice(off, off + width)
                            nc.tensor.matmul(pt[:, sl],
                                             wt0[:, hb * 128:(hb + 1) * 128],
                                             x0[:, sl], start=True, stop=False)
                            nc.tensor.matmul(pt[:, sl],
                                             wt1[:, hb * 128:(hb + 1) * 128],
                                             x1[:, sl], start=False, stop=True)
                        # scan straight out of PSUM: g_t = |g_{t-1} - qn_t|
                        g = gp.tile([128, L], F32, tag="g")
                        nc.vector._custom_dve(_SCAN_OP, out=g[:], in0=pt[:])
                        # keep only g_L
                        nc.scalar.copy(h_all[:, hb * BC + b: hb * BC + b + 1],
                                       g[:, L - 1:L])

            # output projection: out[o, b] = sum_h WHO[h, o] * h_all[h, b] + bias
            for ob in range(O // 128):
                po = pp.tile([128, BC], F32, tag="ps")
                for hb in range(HB):
                    lhs = who[:, hb * O + ob * 128: hb * O + ob * 128 + 128]
                    rhs = h_all[:, hb * BC:(hb + 1) * BC]
                    nc.tensor.matmul(po[:], lhs, rhs,
                                     start=(hb == 0), stop=(hb == HB - 1))
                ot = outp.tile([128, BC], F32, tag=f"ot{ob}")
                nc.scalar.activation(ot[:], po[:], Act.Identity,
                                     bias=bia[:, ob:ob + 1], scale=1.0)
                nc.sync.dma_start(OUT[ob * 128:(ob + 1) * 128, :], ot[:])

    nc.compile()
    _BUILD_CACHE[cache_key] = nc
    return nc


def _prep_inputs(X, W_ih, hh, W_ho, b_ho):
    X = np.asarray(X, dtype=np.float32)
    W_ih = np.asarray(W_ih, dtype=np.float32)
    hh = np.asarray(hh, dtype=np.float32).reshape(-1)
    W_ho = np.asarray(W_ho, dtype=np.float32)
    b_ho = np.asarray(b_ho, dtype=np.float32).reshape(-1)
    c = float(hh[0])
    assert np.allclose(hh, c), "kernel assumes uniform hh (setup_inputs gives 0.99)"
    assert 0.0 < c, "scan rescaling requires positive hh"

    # truncate to the last L steps (see module docstring), restart h=0 there;
    # fold -c^{-(k+1)} into X; fold c^L into W_ho
    L = _suffix_len(c)
    X = X[S - L:]
    tscale = (-np.power(np.float64(c), -(np.arange(L, dtype=np.float64) + 1.0))
              ).astype(np.float32)
    Xs = X * tscale[:, None, None]

    if GEMM_DTYPE == "bf16":
        import ml_dtypes
        gnp = ml_dtypes.bfloat16
        Xs = Xs.astype(gnp)
        WT_h = np.ascontiguousarray(W_ih.T.astype(gnp))                   # [I, H]
    else:
        WT_h = np.ascontiguousarray(W_ih.T)                               # [I, H]
    WHO_h = np.ascontiguousarray((W_ho * np.float32(c ** L)).T)           # [H, O]
    BIA_h = np.ascontiguousarray(b_ho.reshape(O // 128, 128).T)           # [128, 2]

    in_maps = []
    for k in range(NCORES):
        xc = np.ascontiguousarray(
            Xs[:, k * BC:(k + 1) * BC, :].transpose(2, 1, 0))             # [I, BC, L]
        in_maps.append(dict(X=xc, WT=WT_h, WHO=WHO_h, BIA=BIA_h))
    return in_maps, L


def _run(nc, in_maps):
    res = bass_utils.run_bass_kernel_spmd(nc, in_maps, core_ids=list(range(NCORES)))
    return np.concatenate(
        [res.results[k]["out"].T for k in range(NCORES)], axis=0)        # [B, O]


def kernel(X, W_ih, hh, W_ho, b_ho):
    in_maps, L = _prep_inputs(X, W_ih, hh, W_ho, b_ho)
    nc = _build(repeat=1, L=L)
    return _run(nc, in_maps).astype(np.float32)


# revision 14
# speedup vs baseline: 1144.7399x; 1.0466x over previous
"""Trainium2 Bass kernel for nn_AbsDiagNet: out = scan(|p_t + c*h|) @ W_ho.T + b_ho.

Algorithm
---------
reference:  pre = einsum('sbi,hi->sbh', X, W_ih)          # big GEMM
            h_{t+1} = |pre[t] + c * h_t|, h_0 = 0         # serial scan, c=hh[0]
            out = h_S @ W_ho.T + b_ho

Key transform: abs is positively homogeneous, so with g_t := h_t * c^{-t}:
            g_{t+1} = | g_t + pre[t] * c^{-(t+1)} |
The per-step multiply disappears. We fold -c^{-(t+1)} into X on the host, so
the device GEMM directly produces  qn[t] = -pre[t]*c^{-(t+1)},  and the scan is
            g_{t+1} = | g_t - qn[t] |   (ABSOLUTE_DIFF)
which is a single-ALU-op recurrence that a custom DVE op runs at
1 element/lane/cycle along the free axis (two independent chains per
instruction via a SUB_DIM_DONE state-reseed uop). h_S = c^S * g_S is folded
into W_ho.

Suffix truncation: the recurrence is exponentially forgetting -- in h-domain
|dh_S / dh_t| = c^(S-t), so starting the scan at t = S-L with h=0 perturbs the
output by at most ~1.2*c^L (relative, conservative bound incl. the output
projection). L is chosen adaptively from the runtime c so this bound is
<= TRUNC_TOL (for c=0.99: L=768, measured truncation error 8.3e-5 -- 30x
below the bf16 GEMM noise). Cuts GEMM/scan/DMA work to L/S = 37.5%.

Sharding: data-parallel over batch B=128 -> 16 per core x 8 cores. X is
host-transposed per core to [I, Bc, S] so the GEMM needs no on-device
transposes (contraction dim I lands on partitions). GEMM inputs are bf16
(see GEMM_DTYPE); PSUM accumulation and the scan are fp32.
"""
import os
import numpy as np
from contextlib import ExitStack

import concourse.bass as bass
import concourse.tile as tile
from concourse import bacc, mybir
import concourse.bass_utils as bass_utils
import concourse.dve_ops as dve_ops_mod
from concourse.dve_ops import DveOp
from concourse.dve_spec import (
    Spec, Scan as SpecScan, AluOp as DAlu, Src0, Zero, lower as dve_lower,
    _has_src1,
)
from concourse.dve_uop import DveOpSpec

F32 = mybir.dt.float32
F32R = mybir.dt.float32r
BF16 = mybir.dt.bfloat16
Act = mybir.ActivationFunctionType

# GEMM input dtype: "bf16" (end-to-end relerr ~2.3e-3) or "f32r" (~12
# mantissa bits, relerr ~2e-4, PE ~2x slower due to per-MM self-loading
# weights). Both far inside the 2e-2 gate; bf16 keeps the matmuls fully
# hidden under the DVE scan.
GEMM_DTYPE = "bf16"

S, B, I, H, O = 2048, 128, 256, 1024, 256
NCORES = 8
BC = B // NCORES          # 16 batches per core
HB = H // 128             # 8 h-blocks
TRUNC_TOL = 6e-4          # conservative relative-error budget for truncation


def _suffix_len(c: float) -> int:
    """Scan suffix length L: smallest 256-multiple with 1.2*c^L <= TRUNC_TOL.

    The bound is ~6x conservative vs measured (sign cancellation in the
    output projection); at c=0.99 it gives L=768 where the measured
    truncation error is 8.3e-5 -- 30x below the bf16 GEMM noise."""
    import math
    if c >= 1.0 or c <= 0.0:
        return S
    L = math.log(1.2 / TRUNC_TOL) / -math.log(c)
    return min(S, max(512, int(math.ceil(L / 256.0)) * 256))

# --- disable walrus birsim (verification-only; big compile-time cost) -------
_orig_run_command = bass_utils.run_command


def _run_command_nobirsim(argv, **kw):
    argv = ["--enable-birsim=false" if a == "--enable-birsim=true" else a
            for a in argv]
    return _orig_run_command(argv, **kw)


bass_utils.run_command = _run_command_nobirsim


# --- custom DVE op: inclusive scan with state = |state - x| ------------------
def _register_scan_op() -> DveOp:
    name = "ABS_DIFF_SCAN_ANT"
    if name in dve_ops_mod._SUB_OPCODE_FOR_NAME:
        return next(o for o in dve_ops_mod.OPS if o.name == name)
    spec = Spec(body=SpecScan(DAlu.ABSOLUTE_DIFF, Src0, init=Zero))
    row = max(dve_ops_mod._SUB_OPCODE_FOR_NAME.values()) + 1
    assert row < 0x20
    dve_ops_mod._SUB_OPCODE_FOR_NAME[name] = row
    sha = DveOpSpec(name=name, opcode=row, uops=dve_lower(spec, ver="v3"),
                    rd1_en=_has_src1(spec)).sha("v3")
    op = DveOp(name, spec, subdim=False, uops_sha={"v3": sha})
    dve_ops_mod.OPS.append(op)
    dve_ops_mod.CUSTOM_DVE_SPECS[name] = spec
    return op


_SCAN_OP = _register_scan_op()


def _register_scan2_op() -> DveOp:
    """Paired-chain variant: in0/out are [128, 2, L] (2 chains per lane per
    instruction). A third uop re-seeds the scan state to zero when the
    innermost free dim wraps (SUB_DIM_DONE), so the two chains are
    independent. Halves the per-instruction overhead of the scan phase."""
    name = "ABS_DIFF_SCAN2_ANT"
    if name in dve_ops_mod._SUB_OPCODE_FOR_NAME:
        return next(o for o in dve_ops_mod.OPS if o.name == name)
    import copy
    from concourse.dve_uop import Trigger as UTrig
    spec = Spec(body=SpecScan(DAlu.ABSOLUTE_DIFF, Src0, init=Zero))
    seed, steady = dve_lower(spec, ver="v3")
    steady = copy.deepcopy(steady)
    reseed = copy.deepcopy(seed)          # COUNT(1) -> uop 1, no src consumed
    steady.trigger = (UTrig.SRC_TENSOR_DONE, UTrig.SUB_DIM_DONE, UTrig.NONE)
    steady.next_uop = (0, 2, 0)           # end -> IDLE; chain boundary -> reseed
    row = max(dve_ops_mod._SUB_OPCODE_FOR_NAME.values()) + 1
    assert row < 0x20
    dve_ops_mod._SUB_OPCODE_FOR_NAME[name] = row
    spec_obj = DveOpSpec(name=name, opcode=row, uops=[seed, steady, reseed],
                         rd1_en=_has_src1(spec))

    class _HandDveOp(DveOp):
        def compile(self, ver):
            assert ver == "v3", "hand-built uops pinned to v3/TRN2"
            return spec_obj

    op = _HandDveOp(name, spec, subdim=True,
                    uops_sha={"v3": spec_obj.sha("v3")})
    dve_ops_mod.OPS.append(op)
    dve_ops_mod.CUSTOM_DVE_SPECS[name] = spec
    return op


_SCAN2_OP = _register_scan2_op()
SCAN_PAIR = True          # 2 chains per scan instruction (halves DVE overhead)
_BUILD_CACHE: dict = {}


def _build(repeat: int = 1, L: int = S):
    """Build + compile the per-core Bass module (same NEFF on all 8 cores)."""
    cache_key = (repeat, GEMM_DTYPE, L, SCAN_PAIR)
    if cache_key in _BUILD_CACHE:
        return _BUILD_CACHE[cache_key]
    assert L % 256 == 0
    # matmul t-chunks: 512-wide, with a trailing 256 if L % 512
    chunks = [(i * 512, 512) for i in range(L // 512)]
    if L % 512:
        chunks.append((L - 256, 256))
    GD = BF16 if GEMM_DTYPE == "bf16" else F32R
    nc = bacc.Bacc("TRN2", target_bir_lowering=False, debug=False)
    X = nc.dram_tensor("X", [I, BC, L], GD, kind="ExternalInput").ap()
    WT = nc.dram_tensor("WT", [I, H], GD, kind="ExternalInput").ap()
    WHO = nc.dram_tensor("WHO", [H, O], F32, kind="ExternalInput").ap()
    BIA = nc.dram_tensor("BIA", [128, O // 128], F32, kind="ExternalInput").ap()
    OUT = nc.dram_tensor("out", [O, BC], F32, kind="ExternalOutput").ap()

    with tile.TileContext(nc) as tc, ExitStack() as ctx:
        const = ctx.enter_context(tc.tile_pool(name="const", bufs=1))
        xp = ctx.enter_context(tc.tile_pool(name="xp", bufs=4))
        gp = ctx.enter_context(tc.tile_pool(name="gp", bufs=4))
        pp = ctx.enter_context(tc.tile_pool(name="pp", bufs=2, space="PSUM"))
        outp = ctx.enter_context(tc.tile_pool(name="outp", bufs=2))

        wt0 = const.tile([128, H], GD, tag="wt0")
        nc.sync.dma_start(wt0[:], WT[0:128, :])
        wt1 = const.tile([128, H], GD, tag="wt1")
        nc.sync.dma_start(wt1[:], WT[128:256, :])
        who = const.tile([128, HB * O], F32, tag="who")
        nc.sync.dma_start(
            who[:].rearrange("p (g o) -> p g o", g=HB, o=O),
            WHO.rearrange("(g p) o -> p g o", p=128),
        )
        bia = const.tile([128, O // 128], F32, tag="bia")
        nc.sync.dma_start(bia[:], BIA[:])

        for _rep in range(repeat):
            h_all = outp.tile([128, HB * BC], F32, tag="h_all")
            for b in range(BC):
                x0 = xp.tile([128, L], GD, tag="x0")
                nc.sync.dma_start(x0[:], X[0:128, b, :])
                x1 = xp.tile([128, L], GD, tag="x1")
                nc.sync.dma_start(x1[:], X[128:256, b, :])

                if SCAN_PAIR:
                    # chains padded to CP elements in PSUM for bank alignment
                    CP = -(-L // 1024) * 1024
                    hv = h_all[:].rearrange("p (g b) -> p g b", g=HB, b=BC)
                    for hp in range(HB // 2):
                        pt = pp.tile([128, 2 * CP], F32, tag="ps")
                        for j in range(2):
                            hb = 2 * hp + j
                            for off, width in chunks:
                                sl_x = slice(off, off + width)
                                sl_p = slice(j * CP + off, j * CP + off + width)
                                nc.tensor.matmul(
                                    pt[:, sl_p], wt0[:, hb * 128:(hb + 1) * 128],
                                    x0[:, sl_x], start=True, stop=False)
                                nc.tensor.matmul(
                                    pt[:, sl_p], wt1[:, hb * 128:(hb + 1) * 128],
                                    x1[:, sl_x], start=False, stop=True)
                        g = gp.tile([128, 2 * L], F32, tag="g")
                        in_v = pt[:].rearrange("p (c t) -> p c t",
                                               c=2, t=CP)[:, :, 0:L]
                        out_v = g[:].rearrange("p (c t) -> p c t", c=2, t=L)
                        nc.vector._custom_dve(_SCAN2_OP, out=out_v, in0=in_v)
                        nc.scalar.copy(hv[:, 2 * hp:2 * hp + 2, b],
                                       out_v[:, :, L - 1])
                else:
                    for hb in range(HB):
                        # qn for (b, hb): [128 h_sub, L t] accumulated in PSUM
                        pt = pp.tile([128, L], F32, tag="ps")
                        for off, width in chunks:
                            sl = slice(off, off + width)
                            nc.tensor.matmul(pt[:, sl],
                                             wt0[:, hb * 128:(hb + 1) * 128],
                                             x0[:, sl], start=True, stop=False)
                            nc.tensor.matmul(pt[:, sl],
                                             wt1[:, hb * 128:(hb + 1) * 128],
                                             x1[:, sl], start=False, stop=True)
                        # scan straight out of PSUM: g_t = |g_{t-1} - qn_t|
                        g = gp.tile([128, L], F32, tag="g")
                        nc.vector._custom_dve(_SCAN_OP, out=g[:], in0=pt[:])
                        # keep only g_L
                        nc.scalar.copy(h_all[:, hb * BC + b: hb * BC + b + 1],
                                       g[:, L - 1:L])

            # output projection: out[o, b] = sum_h WHO[h, o] * h_all[h, b] + bias
            for ob in range(O // 128):
                po = pp.tile([128, BC], F32, tag="ps")
                for hb in range(HB):
                    lhs = who[:, hb * O + ob * 128: hb * O + ob * 128 + 128]
                    rhs = h_all[:, hb * BC:(hb + 1) * BC]
                    nc.tensor.matmul(po[:], lhs, rhs,
                                     start=(hb == 0), stop=(hb == HB - 1))
                ot = outp.tile([128, BC], F32, tag=f"ot{ob}")
                nc.scalar.activation(ot[:], po[:], Act.Identity,
                                     bias=bia[:, ob:ob + 1], scale=1.0)
                nc.sync.dma_start(OUT[ob * 128:(ob + 1) * 128, :], ot[:])

    nc.compile()
    _BUILD_CACHE[cache_key] = nc
    return nc


def _prep_inputs(X, W_ih, hh, W_ho, b_ho):
    X = np.asarray(X, dtype=np.float32)
    W_ih = np.asarray(W_ih, dtype=np.float32)
    hh = np.asarray(hh, dtype=np.float32).reshape(-1)
    W_ho = np.asarray(W_ho, dtype=np.float32)
    b_ho = np.asarray(b_ho, dtype=np.float32).reshape(-1)
    c = float(hh[0])
    assert np.allclose(hh, c), "kernel assumes uniform hh (setup_inputs gives 0.99)"
    assert 0.0 < c, "scan rescaling requires positive hh"

    # truncate to the last L steps (see module docstring), restart h=0 there;
    # fold -c^{-(k+1)} into X; fold c^L into W_ho
    L = _suffix_len(c)
    X = X[S - L:]
    tscale = (-np.power(np.float64(c), -(np.arange(L, dtype=np.float64) + 1.0))
              ).astype(np.float32)
    Xs = X * tscale[:, None, None]

    if GEMM_DTYPE == "bf16":
        import ml_dtypes
        gnp = ml_dtypes.bfloat16
        Xs = Xs.astype(gnp)
        WT_h = np.ascontiguousarray(W_ih.T.astype(gnp))                   # [I, H]
    else:
        WT_h = np.ascontiguousarray(W_ih.T)                               # [I, H]
    WHO_h = np.ascontiguousarray((W_ho * np.float32(c ** L)).T)           # [H, O]
    BIA_h = np.ascontiguousarray(b_ho.reshape(O // 128, 128).T)           # [128, 2]

    in_maps = []
    for k in range(NCORES):
        xc = np.ascontiguousarray(
            Xs[:, k * BC:(k + 1) * BC, :].transpose(2, 1, 0))             # [I, BC, L]
        in_maps.append(dict(X=xc, WT=WT_h, WHO=WHO_h, BIA=BIA_h))
    return in_maps, L


def _run(nc, in_maps):
    res = bass_utils.run_bass_kernel_spmd(nc, in_maps, core_ids=list(range(NCORES)))
    return np.concatenate(
        [res.results[k]["out"].T for k in range(NCORES)], axis=0)        # [B, O]


def kernel(X, W_ih, hh, W_ho, b_ho):
    in_maps, L = _prep_inputs(X, W_ih, hh, W_ho, b_ho)
    nc = _build(repeat=1, L=L)
    return _run(nc, in_maps).astype(np.float32)
